# revision 61
# baseline (speedup 1.0000x reference)
"""Single-head causal attention on 8 TRN2 NeuronCores.

Problem: x [8, 2048, 1024] f32, Wq/Wk/Wv [1024, 64] f32.
  q = x @ Wq ; k = x @ Wk ; v = x @ Wv        (per batch)
  out = softmax(causal(q k^T / 8)) @ v        [8, 2048, 64]

Sharding: data-parallel over batch -- core i handles batch element i.
No collectives needed.

Per-core kernel (bf16 compute, f32 accumulate), 128-token-tile pipeline
tuned against the TimelineSim cost model (59.8us vs 76.2us baseline):

  1. DMA carries ONLY payload (x loads 23.3us = roofline, W 2.2us,
     stores 1.5us): every transpose happens on-chip.  W loads use the
     natural row-contiguous layout (2KB descriptors, no sub-512B DMA
     penalty); the d-contraction is chunked INTERLEAVED (chunk a =
     {d : d = 8p + a}) so that layout needs no weight transpose; ACT
     packs [Wq|Wk|Wv] to one bf16 tile while otherwise idle.
  2. x streams in 256-token groups; each 128-tile is cast f32->bf16
     into the interleaved (a, j) layout (2/3 on gpsimd, 1/3 on DVE),
     transposed on the PE (8x [128,128] identity matmuls -> PSUM bf16,
     double-buffered), and copied to the x^T SBUF pool by DVE.
     Front (cast/transpose/copy) and back (projection/drains) stages
     are emitted with a 2-tile skew so the PE round-trips never expose
     their latency on the in-order DVE queue.
  3. Projections are x-stationary: lhsT = x^T tile-chunk, moving
     rhs = [Wq|Wk|Wv] (192 wide) -> PSUM [t,192] in 8 matmuls/tile
     (1536 PE cycles vs 2048 for the W-stationary form).  ONE DVE copy
     drains Q|K|V to the persistent vqk tile whose col 192 is 1.0 (the
     softmax-denominator row for PV's 65-row lhsT).
  4. Q,K are re-transposed on the PE ([128,64] -> [64,128], 2x128
     cycles/tile) into qkT; a DMA-xbar alternative loses badly here:
     xbar traffic serializes against the x stream on the single
     DMA_ENGINES resource and its waits poison the in-order HWDGE
     queues.
  5. Attention in 256-row q-blocks (8 blocks; they gate on 2-tile qkT
     groups and their S-pair PSUM tiles are a single bank):
     S^T[tk,tq] = K^T_tile.T @ Q^T (contraction h=64), exp on ACT in
     k-tile PAIRS (one [128,<=512] instruction amortizes ACT's ~185ns
     access latency), causal diagonal via a multiplicative 0/1 bf16
     mask on gpsimd, PV accumulates out^T[65,tq] += vqk[ki].T @ P^T
     with row 64 the denominators.  Blocks are emitted in STEPS
     (S/exp of pair p+1, then PV of pair p) interleaved with tile work
     so the exp-gated PVs never head-of-line block the PE queue.
  6. Output: PSUM -> bf16 SBUF (ACT), PE-transpose back to [tq,65],
     DVE copy+reciprocal, gpsimd rescale, bf16 store (f32 upcast
     happens host-side after the gather).

Scheduling invariants (cost model): every engine SEQ is in-order and
an explicitly-emitted semaphore wait blocks everything behind it, so
per-queue emission order must track true data-arrival order; all DMA
shares one serialized 360GB/s resource (FIFO), so only loads/stores
may use it; PSUM is 8 bank-granular slots: pxt 2 + psS 3 + pout 1 +
psmall(proj/qk/pot) 2.
"""
import numpy as np

import concourse.bass as bass
import concourse.tile as tile
from concourse import bacc, mybir
from concourse.bass_utils import run_bass_kernel_spmd

B, T, D, H = 8, 2048, 1024, 64
P = 128            # partitions / tile edge
ND = D // P        # 8 d-chunks (interleaved: chunk a = {d : d = 8p + a})
NT = T // P        # 16 token tiles
NB = T // 512      # 4 q-blocks of 512 rows
VA = 80            # v_aug padded k-tile stride

FP32 = mybir.dt.float32
BF16 = mybir.dt.bfloat16

_compiled = None

# schedule tunables (swept via TimelineSim)
TUNE = {
    "skew": 2,          # tiles between front(i) and back(i)
    "cast_mod": 3,      # cast on DVE when i % cast_mod == 0 else gpsimd
    "step1_until": 10,  # 1 attention step per tile slot before this tile
    "blk_start": 8,     # first tile index that starts a block
    "tail_burst": 4,    # steps right after appending blocks 6,7
    "xload_bufs": 6,
    "xbt_bufs": 4,
    "ptp_bufs": 3,
    "psS_bufs": 3,
    "steps_off_from": 99,  # no in-loop attention steps from this tile on
    "xbar_from": 99,    # tiles >= this transpose x via DMA xbar (the
                        # loads have drained by then, so the shared DMA
                        # FIFO is idle and the SP queue has nothing left
                        # to head-of-line block)
}


def _build():
    nc = bacc.Bacc("TRN2", target_bir_lowering=False, debug=False, num_devices=8)

    x_d = nc.dram_tensor("x", [T, D], FP32, kind="ExternalInput").ap()
    wq_d = nc.dram_tensor("Wq", [D, H], FP32, kind="ExternalInput").ap()
    wk_d = nc.dram_tensor("Wk", [D, H], FP32, kind="ExternalInput").ap()
    wv_d = nc.dram_tensor("Wv", [D, H], FP32, kind="ExternalInput").ap()
    out_d = nc.dram_tensor("out", [T, H], BF16, kind="ExternalOutput").ap()

    with tile.TileContext(nc) as tc:
        _kernel(tc, out_d, x_d, wq_d, wk_d, wv_d)

    nc.compile()
    return nc


def _kernel(tc, out_d, x_d, wq_d, wk_d, wv_d):
    nc = tc.nc
    from contextlib import ExitStack

    ctx = ExitStack()
    with ctx:
        const = ctx.enter_context(tc.tile_pool(name="const", bufs=1))
        wstage = ctx.enter_context(tc.tile_pool(name="wstage", bufs=3))
        xload = ctx.enter_context(tc.tile_pool(name="xload", bufs=TUNE["xload_bufs"]))
        xbtp = ctx.enter_context(tc.tile_pool(name="xbtp", bufs=TUNE["xbt_bufs"]))
        xtp = ctx.enter_context(tc.tile_pool(name="xtp", bufs=1))
        qkp = ctx.enter_context(tc.tile_pool(name="qkp", bufs=1))
        vsb = ctx.enter_context(tc.tile_pool(name="vsb", bufs=1))
        ptp = ctx.enter_context(tc.tile_pool(name="ptp", bufs=TUNE["ptp_bufs"]))
        obp = ctx.enter_context(tc.tile_pool(name="obp", bufs=2))
        osbp = ctx.enter_context(tc.tile_pool(name="osbp", bufs=2))
        recp = ctx.enter_context(tc.tile_pool(name="recp", bufs=2))
        pxt = ctx.enter_context(tc.tile_pool(name="pxt", bufs=2, space="PSUM"))
        psS = ctx.enter_context(tc.tile_pool(name="psS", bufs=TUNE["psS_bufs"], space="PSUM"))
        pout = ctx.enter_context(tc.tile_pool(name="pout", bufs=1, space="PSUM"))
        psmall = ctx.enter_context(tc.tile_pool(name="psmall", bufs=2, space="PSUM"))

        # ---- loads: first x group, then weights, then remaining x ----
        # (x group 0 first so the cast/transpose pipeline starts ~2us
        # earlier; W only gates the projections, which queue behind.)
        x_r = x_d.rearrange("(g u p) d -> g p u d", p=P, u=2)
        xfs = {}

        def load_x(g):
            xf = xload.tile([P, 2, D], FP32, tag="xf", name=f"xf{g}")
            nc.sync.dma_start(out=xf[:], in_=x_r[g])
            xfs[g] = xf

        load_x(0)

        # Weight loads use the natural row-contiguous layout (2KB
        # descriptors; no sub-512B DMA penalty); chunk a of the interleaved
        # contraction is the partition-slice [:, a, :], so no weight
        # transpose is ever needed.
        w_all = const.tile([P, ND, 3 * H], BF16)   # [Wq | Wk | Wv] per slot
        wnats = []
        for w_dram, name in ((wq_d, "wq"), (wk_d, "wk"), (wv_d, "wv")):
            wn = wstage.tile([P, ND, H], FP32, tag="wstage", name=f"stg_{name}")
            nc.sync.dma_start(out=wn[:], in_=w_dram.rearrange(
                "(p a) h -> p a h", p=P))
            wnats.append(wn)
        for g in range(1, NT // 2):
            load_x(g)

        # Weight pack on ACT: its queue is empty until the first exps
        # (~12us), so waiting on the W loads here blocks nothing.
        for j, wn in enumerate(wnats):
            nc.scalar.copy(out=w_all[:, :, j * H:(j + 1) * H], in_=wn[:])

        # ---- constants ----
        ident_bf = const.tile([P, P], BF16)
        from concourse.masks import make_identity
        make_identity(nc, ident_bf[:])

        # 0/1 upper-triangular (incl. diagonal) bf16 mask in [tk, tq]
        # orientation: valid when tq >= tk (col >= row).
        tri01 = const.tile([P, P], BF16)
        nc.gpsimd.memset(tri01[:], 1.0)
        nc.gpsimd.affine_select(
            out=tri01[:], in_=tri01[:],
            compare_op=mybir.AluOpType.is_ge,
            fill=0.0, base=0,
            pattern=[[1, P]], channel_multiplier=-1)

        # Per-tile projection results, persistent: [:, i, 0:64] = Q,
        # [64:128] = K (both t-major, transposed later on the PE),
        # [128:192] = V, col 192 = 1.0 (softmax-denominator row for the
        # PV matmul's 65-row lhsT).  One DVE copy drains all three.
        vqk = vsb.tile([P, NT, 200], BF16)
        nc.gpsimd.memset(vqk[:, :, 3 * H:3 * H + 1], 1.0)


        # persistent SBUF state
        xT = xtp.tile([P, ND, T], BF16)      # x^T, interleaved chunks
        # Q^T/K^T ([:, 0] = Q^T, [:, 1] = K^T), h on partitions 0:64 --
        # produced by per-tile PE transposes (the DMA-xbar alternative
        # serializes against the x stream on the shared DMA engines and
        # poisons the in-order HWDGE queues with its waits).
        qkT = qkp.tile([H, 2, T], BF16)

        # ---- per-tile pipeline, split front/back and emitted with a
        # one-tile skew (front(i+1) before back(i)) so the PE round-trips
        # inside a tile (transpose->copy->proj->drain->qk-transpose) never
        # expose their latency on the in-order DVE queue.
        def tile_front(i):
            g, u = divmod(i, 2)
            # cast f32 -> bf16 into the interleaved (a, j) layout:
            # element d of the tile lands at [a = d % 8, j = d // 8].
            # Every third tile casts on the (otherwise idle) gpsimd so the
            # DVE keeps up with its PSUM-drain copies.
            xbt = xbtp.tile([P, ND, P], BF16, tag="xbt", name=f"xbt{i}")
            cast_eng = nc.gpsimd if i % 3 == 2 else nc.vector
            cast_eng.tensor_copy(
                out=xbt[:].rearrange("p a j -> p j a"), in_=xfs[g][:, u, :])
            if i >= TUNE["xbar_from"]:
                # late tiles: one xbar transpose of the already-interleaved
                # stage -- row a*128+j lands at partition j, slot a, which
                # IS the interleaved x^T layout.  448ns of otherwise-idle
                # DMA instead of 1024 PE cycles + a 658ns DVE copy.
                nc.sync.dma_start(out=xT[:, :, i * P:(i + 1) * P],
                                  in_=xbt[:], transpose=True)
            else:
                # PE transposes: chunk a -> x^T[:, a, tile i]
                px = pxt.tile([P, ND, P], BF16, tag="pxt", name=f"px{i}")
                for a in range(ND):
                    nc.tensor.transpose(px[:, a, :], xbt[:, a, :],
                                        ident_bf[:])
                nc.vector.tensor_copy(
                    out=xT[:, :, i * P:(i + 1) * P], in_=px[:])

        def tile_back(i):
            # projection: x^T-stationary, W moving (192 wide)
            ps_p = psmall.tile([P, 3 * H], FP32, tag="small", name=f"psp{i}")
            for a in range(ND):
                nc.tensor.matmul(ps_p[:], xT[:, a, i * P:(i + 1) * P],
                                 w_all[:, a, :],
                                 start=(a == 0), stop=(a == ND - 1))
            # single drain: Q|K|V -> vqk (bf16)
            nc.vector.tensor_copy(out=vqk[:, i, 0:3 * H], in_=ps_p[:])
            # Q,K -> PE transposes -> qkT
            pqk = psmall.tile([H, 2, P], BF16, tag="small", name=f"pqk{i}")
            for u in range(2):
                nc.tensor.transpose(pqk[:, u, :], vqk[:, i, u * H:(u + 1) * H],
                                    ident_bf[:])
            nc.vector.tensor_copy(out=qkT[:, :, i * P:(i + 1) * P], in_=pqk[:])

        # ---- attention ----
        stores = []

        def diag(b, ki):
            return 4 * b <= ki < 4 * b + 4

        class AttnBlock:
            """Attention for one 256-row q-block (8 blocks total), emitted
            in STEPS so the exp-gated PV matmuls interleave with later
            tiles' PE work instead of head-of-line blocking the in-order
            PE queue.  256-wide blocks gate on 2-tile qkT groups (tight
            streaming, small tail) and their S-pair PSUM tiles are a
            single bank."""

            QW = 256

            AHEAD = 1   # S/exp emitted this many pairs before their PV

            def __init__(self, j):
                self.j = j
                self.qlo = self.QW * j
                self.pairs = [(2 * p, 2 * p + 1) for p in range(j + 1)]
                self.ps_o = pout.tile([H + 1, self.QW], FP32, tag="pout",
                                      name=f"pso{j}")
                self.idx = 0
                self.pending = [self.s_exp(pr) for pr in self.pairs[:self.AHEAD]]
                self.ob = None

            def s_exp(self, pr):
                j, qlo, QW = self.j, self.qlo, self.QW
                k0, k1 = pr
                w0 = max(0, k0 * P - qlo)
                w1 = max(0, k1 * P - qlo)
                ps = psS.tile([P, 2 * QW], FP32, tag="psS",
                              name=f"psS{j}_{k0}")
                pt = ptp.tile([P, 2 * QW], BF16, tag="pt", name=f"pt{j}_{k0}")
                for ki, w, pos in ((k0, w0, 0), (k1, w1, QW)):
                    nc.tensor.matmul(
                        ps[:, pos + w:pos + QW],
                        qkT[:, 1, ki * P:(ki + 1) * P],
                        qkT[:, 0, qlo + w:qlo + QW],
                        start=True, stop=True)
                if j < 2:
                    # fresh PSUM slots: exp only over written regions
                    for ki, w, pos in ((k0, w0, 0), (k1, w1, QW)):
                        if w < QW:
                            nc.scalar.activation(
                                out=pt[:, pos + w:pos + QW],
                                in_=ps[:, pos + w:pos + QW],
                                func=mybir.ActivationFunctionType.Exp,
                                scale=0.125)
                else:
                    # one wide exp; the [QW, QW+w1) gap holds stale
                    # (finite) values from an earlier pair and is never
                    # read by PV.
                    nc.scalar.activation(
                        out=pt[:, w0:2 * QW], in_=ps[:, w0:2 * QW],
                        func=mybir.ActivationFunctionType.Exp,
                        scale=0.125)
                for ki, w, pos in ((k0, w0, 0), (k1, w1, QW)):
                    if 2 * j <= ki <= 2 * j + 1 and w < QW:
                        # diagonal k-tile: zero the strictly-lower triangle
                        nc.gpsimd.tensor_mul(pt[:, pos + w:pos + w + P],
                                             pt[:, pos + w:pos + w + P],
                                             tri01[:])
                return pt, w0, w1

            def done(self):
                return self.idx >= len(self.pairs)

            def step(self):
                """Emit S/exp of pair idx+1 (pipeline-ahead), then PV of
                pair idx.  On the last step, drain ps_o to bf16 SBUF."""
                j, idx, pairs, QW = self.j, self.idx, self.pairs, self.QW
                k0, k1 = pairs[idx]
                if idx + self.AHEAD < len(pairs):
                    self.pending.append(self.s_exp(pairs[idx + self.AHEAD]))
                pt, w0, w1 = self.pending.pop(0)
                for ki, w, pos in ((k0, w0, 0), (k1, w1, QW)):
                    if w >= QW:
                        continue
                    nc.tensor.matmul(
                        self.ps_o[:, w:QW], vqk[:, ki, 2 * H:3 * H + 1],
                        pt[:, pos + w:pos + QW],
                        start=(idx == 0 and ki == k0),
                        stop=(idx == len(pairs) - 1 and ki == k1))
                self.idx += 1
                if self.done():
                    self.ob = obp.tile([H + 1, self.QW], BF16, tag="ob",
                                       name=f"ob{j}")
                    nc.scalar.copy(out=self.ob[:], in_=self.ps_o[:])

        def out_stage(j, ob):
            QW = AttnBlock.QW
            nj = QW // P
            pot = psmall.tile([P, nj, VA], BF16, tag="small", name=f"pot{j}")
            for u in range(nj):
                nc.tensor.transpose(pot[:, u, 0:H + 1],
                                    ob[:, u * P:(u + 1) * P],
                                    ident_bf[0:H + 1, 0:H + 1])
            # Only TWO DVE queue entries here (copy + recip) -- they park in
            # the 4-deep wait queue without stalling DVE.SEQ for the
            # streaming casts behind them; the rescales run on idle gpsimd
            # from SBUF.
            ot = osbp.tile([P, nj, H + 1], BF16, tag="ot", name=f"ot{j}")
            nc.vector.tensor_copy(out=ot[:], in_=pot[:, :, 0:H + 1])
            rec = recp.tile([P, nj], FP32, tag="rec", name=f"rec{j}")
            nc.vector.reciprocal(rec[:], ot[:, :, H])
            osb = osbp.tile([P, nj, H], BF16, tag="osb", name=f"osb{j}")
            for u in range(nj):
                nc.gpsimd.tensor_scalar_mul(osb[:, u, :], ot[:, u, 0:H],
                                            rec[:, u:u + 1])
            stores.append(
                (out_d.rearrange("(b u p) h -> b p u h", p=P, u=nj)[j], osb))

        # Emission tracks data arrival: block j (256 q-rows) starts one
        # tile after its last q-tile's back-stage; pair-steps interleave
        # with tile work so the exp-gated PVs never clump ahead of later
        # tiles' PE work; out stages weave between the late blocks' steps.
        active = []
        finished = []
        staged = 0
        started = 0

        def run_steps(budget):
            nonlocal staged
            n = 0
            while active and n < budget:
                blk = active[0]
                blk.step()
                n += 1
                if blk.done():
                    finished.append(active.pop(0))
            # stage a finished block once its successor has finished too
            if staged < len(finished) - 1:
                blk = finished[staged]
                out_stage(blk.j, blk.ob)
                staged += 1

        SKEW = TUNE["skew"]
        for i in range(NT):
            if i >= SKEW:
                tile_back(i - SKEW)
            tile_front(i)
            j = (i - TUNE["blk_start"]) // 2
            if i >= TUNE["blk_start"] and (i - TUNE["blk_start"]) % 2 == 0 and j <= 7:
                active.append(AttnBlock(j))
                started = j + 1
            if i < TUNE["steps_off_from"]:
                run_steps(1 if i < TUNE["step1_until"] else 2)
        for i in range(NT - SKEW, NT):
            tile_back(i)
        for j in range(started, 8):
            active.append(AttnBlock(j))
            run_steps(TUNE["tail_burst"])
        while active:
            run_steps(2)
        for blk in finished[staged:]:
            out_stage(blk.j, blk.ob)

        for dst, osb in stores:
            nc.sync.dma_start(out=dst, in_=osb[:])


def _run(inputs, trace=False, **kw):
    global _compiled
    if _compiled is None:
        _compiled = _build()
    nc = _compiled
    x = np.ascontiguousarray(inputs["x"], dtype=np.float32)
    wq = np.ascontiguousarray(inputs["Wq"], dtype=np.float32)
    wk = np.ascontiguousarray(inputs["Wk"], dtype=np.float32)
    wv = np.ascontiguousarray(inputs["Wv"], dtype=np.float32)
    in_maps = [
        {"x": np.ascontiguousarray(x[i]), "Wq": wq, "Wk": wk, "Wv": wv}
        for i in range(B)
    ]
    res = run_bass_kernel_spmd(nc, in_maps, core_ids=list(range(B)),
                               trace=trace, **kw)
    out = np.stack(
        [np.asarray(res.results[i]["out"]).astype(np.float32) for i in range(B)],
        axis=0)
    return out, res


def kernel(x, Wq, Wk, Wv):
    out, _ = _run({"x": x, "Wq": Wq, "Wk": Wk, "Wv": Wv})
    return out
